# revision 26
# baseline (speedup 1.0000x reference)
"""Doc2vec-style embedding lookup kernel for 8 Trainium2 NeuronCores.

Computation (per batch row b):
    h[b,:]      = D[docs[b],:] + sum_c W[ctxs[b,c],:]          # [B, DIM]
    scores[b,k] = sum_d h[b,d] * WP[d, targets[b,k]]           # [B, K1]

Strategy: pure batch data-parallel over 8 cores (2048 rows each), tables
replicated.  On the host we fold W, D and WP^T into ONE row-padded bf16
table (rows padded 100 -> 128 bf16 = 256B) and pack all per-row indices
into a single [B, 15] int32 array (8 ctx, 1 doc, 6 target indices,
pre-offset into the fused table).  On device each core runs 240
INDIRECT1D gathers (one 128-row gather per (tile, j) pair), then a
strided 9-way vector reduce for h and fused multiply-reduce for the
scores, all in bf16 (fp32 accumulate on DVE).

Measured structural limits of this environment (ntff-profiled):
 - The wall is the GpSimd (Pool) engine: each DMA_INDIRECT op holds
   the engine ~1108ns (SWDGE ucode launch; descgen for 128 descs is
   only ~0.5ns/desc on top) plus ~310ns SEQ dispatch gap -> 240 ops
   x ~1.42us = ~340us serial.  Actual DMA transfer time is only
   ~128us (16.7ns per 4x256B packet) and hides completely under the
   launch serialization ("dynamic dma busy 98%" counts queue-occupied
   time, not transfers -- software_dynamic_dma_active_time is truth).
 - One offset per partition per op is a hard ceiling (128 rows/op):
   multi-offset vector-indirect gathers (offset AP [128, n]) are NOT
   honored by this ucode -- a 2D dest fuses into one descriptor per
   partition (gathers n contiguous rows from offset 0: wrong), and a
   3D dest produces partial writes (NaN) with BOTH 200B-misaligned
   (~25us/op) and 256B-aligned padded-slab (~39us/op) layouts
   (SUNDA-generation DGE only supports indirection_dim=0).
 - Indirect DMA is SWDGE-only: routing the same InstDMACopy+dynamic_ap
   through the idle SP/Activation HWDGE dynamic queues (qActDynamicHW)
   compiles but crashes the runtime -- RTL DGE cannot do indirection.
 - Splitting ops across qPoolDynamic1 (num_swdge_queues=2) executes
   correctly but is ~18% slower (one Q7 pair services both queues);
   single_packet=True is accepted and harmless but changes nothing.
 - The Anthropic extended-ISA ucode ops (dma_gather/ap_gather/
   scatter_add, int16-indexed, 4 parallel SWDGE queue pairs) would
   break the launch bottleneck but crash this runtime (stock NRT
   ucode without the extended opcodes): NRT_EXEC_UNIT_UNRECOVERABLE,
   while the same program passes CoreSim bit-exactly.
"""

import sys

sys.path.insert(0, "/opt/trn_rl_repo")

import numpy as np
import ml_dtypes

# ---- problem constants (hardcoded; kernel.py must be self-contained) ----
B = 16384  # batch
CTX = 8  # context words per row
K1 = 6  # targets per row
DIM = 100  # embedding dim
NW = 200001  # word rows (incl. padding row)
ND = 1000000  # doc rows
NCORES = 8
BPC = B // NCORES  # 2048 batch rows per core
P = 128  # SBUF partitions
TILES = BPC // P  # 16 tiles of 128 rows per core
JPT = CTX + 1 + K1  # 15 gathered rows per batch row
DPAD = 128  # padded row length (256B in bf16)

_CACHE: dict = {}


def _build_program(nrows=NW + ND + NW):
    import concourse.bass as bass
    import concourse.bacc as bacc
    import concourse.mybir as mybir
    import concourse.tile as tile

    bf16 = mybir.dt.bfloat16
    nc = bacc.Bacc("TRN2", target_bir_lowering=False, debug=False,
                   num_devices=NCORES)
    idx_d = nc.dram_tensor("idx", [BPC, JPT], mybir.dt.int32,
                           kind="ExternalInput")
    tab_d = nc.dram_tensor("table", [nrows, DPAD], bf16,
                           kind="ExternalInput")
    out_d = nc.dram_tensor("scores", [BPC, K1], mybir.dt.float32,
                           kind="ExternalOutput")

    with tile.TileContext(nc) as tc:
        with tc.tile_pool(name="sb", bufs=1) as sb, \
             tc.tile_pool(name="scr", bufs=4) as scr:
            idx_sb = sb.tile([P, TILES * JPT], mybir.dt.int32)
            # Load tile 0's indices first so the gather pipeline starts
            # without waiting for the whole index array.
            idx_r = idx_d.ap().rearrange("(t p) j -> p t j", p=P)
            idx_v = idx_sb[:].rearrange("p (t j) -> p t j", t=TILES)
            nc.sync.dma_start(out=idx_v[:, 0:1], in_=idx_r[:, 0:1])
            nc.sync.dma_start(out=idx_v[:, 1:TILES], in_=idx_r[:, 1:TILES])
            scores_sb = sb.tile([P, TILES * K1], mybir.dt.float32)
            # ALL gather destinations statically allocated (61KB/partition
            # fits SBUF easily).  With no buffer reuse there are no
            # write-after-read waits on the Pool engine, so the 240 SWDGE
            # launches (the ~1us fixed cost each is the structural floor
            # of this kernel) issue back-to-back with minimal gaps.
            Gall = sb.tile([P, TILES * (CTX + 1) * DPAD], bf16)
            Gtall = sb.tile([P, TILES * K1 * DPAD], bf16)
            for t in range(TILES):
                # HW indirect DMA supports ONE offset per partition, so we
                # gather the 15 rows of this 128-row batch tile with 15 ops.
                # ctx+doc rows (j<9) and target rows (j>=9) go to separate
                # regions so the h-reduce only waits on the first 9 gathers.
                G = Gall[:, t * (CTX + 1) * DPAD:(t + 1) * (CTX + 1) * DPAD]
                Gt = Gtall[:, t * K1 * DPAD:(t + 1) * K1 * DPAD]
                for j in range(JPT):
                    dst = G[:, j * DPAD:(j + 1) * DPAD] if j <= CTX \
                        else Gt[:, (j - CTX - 1) * DPAD:(j - CTX) * DPAD]
                    nc.gpsimd.indirect_dma_start(
                        out=dst,
                        out_offset=None,
                        in_=tab_d.ap(),
                        in_offset=bass.IndirectOffsetOnAxis(
                            ap=idx_sb[:, t * JPT + j:t * JPT + j + 1],
                            axis=0,
                        ),
                    )
                if t % 2 == 0:
                    continue
                # Batch the DVE work over tile PAIRS (4D APs): halves the
                # vector-op count and the cross-engine sync events that
                # interleave with the Pool launch stream.
                t0 = t - 1
                G2 = Gall[:, t0 * (CTX + 1) * DPAD:(t + 1) * (CTX + 1) * DPAD]
                Gt2 = Gtall[:, t0 * K1 * DPAD:(t + 1) * K1 * DPAD]
                G3 = G2.rearrange("p (u j d) -> p u d j", u=2, j=CTX + 1,
                                  d=DPAD)
                h = scr.tile([P, 2 * DIM], bf16, tag="h")
                # h = sum of the 8 ctx rows + 1 doc row (slabs j=0..8).
                # DVE accumulates in fp32 internally; bf16 storage only
                # rounds the 9-term sum once (rel ~2^-9, well under the
                # 2e-2 gate) and keeps the 2x 16-bit DVE mode.
                h4 = h[:].rearrange("p (u d) -> p u d", u=2)
                with nc.allow_low_precision(reason="bf16 h, fp32 internal"):
                    nc.vector.tensor_reduce(
                        out=h4, in_=G3[:, :, 0:DIM, 0:CTX + 1],
                        axis=mybir.AxisListType.X, op=mybir.AluOpType.add,
                    )
                # prod[p, u, k, d] = h[p, u, d] * tgt[p, u, k, d]; reduce d
                prod = scr.tile([P, 2 * K1 * DIM], bf16, tag="prod")
                tgt = Gt2.rearrange("p (u k d) -> p u k d", u=2, k=K1)
                nc.vector.tensor_tensor(
                    out=prod[:].rearrange("p (u k d) -> p u k d", u=2, k=K1),
                    in0=tgt[:, :, :, 0:DIM],
                    in1=h4.unsqueeze(2).to_broadcast([P, 2, K1, DIM]),
                    op=mybir.AluOpType.mult,
                )
                nc.vector.tensor_reduce(
                    out=scores_sb[:, t0 * K1:(t + 1) * K1].rearrange(
                        "p (u k) -> p u k", u=2),
                    in_=prod[:].rearrange("p (u k d) -> p u k d", u=2, k=K1),
                    axis=mybir.AxisListType.X, op=mybir.AluOpType.add,
                )
                # Flush finished quarters early so the final output DMA
                # isn't one serialized tail after the last vector op.
                if (t + 1) % (TILES // 4) == 0:
                    lo, hi = t + 1 - TILES // 4, t + 1
                    nc.sync.dma_start(
                        out=out_d.ap().rearrange("(t p) k -> p t k", p=P)
                            [:, lo:hi],
                        in_=scores_sb[:].rearrange("p (t k) -> p t k", t=TILES)
                            [:, lo:hi],
                    )
    nc.compile()
    return nc


def _get_program():
    if "nc" not in _CACHE:
        _CACHE["nc"] = _build_program()
    return _CACHE["nc"]


def _pack_inputs(ctxs, docs, targets, D, W, WP):
    """Fuse tables into one 256B-row bf16 table; pack indices to [B,15]."""
    table = np.zeros((NW + ND + NW, DPAD), dtype=ml_dtypes.bfloat16)
    table[:NW, :DIM] = np.asarray(W, dtype=np.float32)
    table[NW:NW + ND, :DIM] = np.asarray(D, dtype=np.float32)
    table[NW + ND:, :DIM] = np.asarray(WP, dtype=np.float32).T
    idx = np.empty((B, JPT), dtype=np.int32)
    idx[:, :CTX] = np.asarray(ctxs, dtype=np.int32)
    idx[:, CTX] = np.asarray(docs, dtype=np.int32) + NW
    idx[:, CTX + 1:] = np.asarray(targets, dtype=np.int32) + (NW + ND)
    return table, idx


def kernel(ctxs, docs, targets, D, W, WP, _trace=False):
    from concourse.bass_utils import run_bass_kernel_spmd

    table, idx = _pack_inputs(ctxs, docs, targets, D, W, WP)
    nc = _get_program()
    in_maps = [
        {"idx": np.ascontiguousarray(idx[c * BPC:(c + 1) * BPC]),
         "table": table}
        for c in range(NCORES)
    ]
    res = run_bass_kernel_spmd(nc, in_maps, core_ids=list(range(NCORES)),
                               trace=_trace)
    out = np.concatenate([res.results[c]["scores"] for c in range(NCORES)],
                         axis=0)
    if _trace:
        return out, res
    return out



# revision 27
# speedup vs baseline: 1.0089x; 1.0089x over previous
"""Doc2vec-style embedding lookup kernel for 8 Trainium2 NeuronCores.

Computation (per batch row b):
    h[b,:]      = D[docs[b],:] + sum_c W[ctxs[b,c],:]          # [B, DIM]
    scores[b,k] = sum_d h[b,d] * WP[d, targets[b,k]]           # [B, K1]

Strategy: pure batch data-parallel over 8 cores (2048 rows each), tables
replicated.  On the host we fold W, D and WP^T into ONE row-padded bf16
table (rows padded 100 -> 128 bf16 = 256B) and pack all per-row indices
into a single [B, 15] int32 array (8 ctx, 1 doc, 6 target indices,
pre-offset into the fused table).  On device each core runs 240
INDIRECT1D gathers (one 128-row gather per (tile, j) pair), then a
strided 9-way vector reduce for h and fused multiply-reduce for the
scores, all in bf16 (fp32 accumulate on DVE).

Measured structural limits of this environment (ntff-profiled):
 - The wall is the GpSimd (Pool) engine: each DMA_INDIRECT op holds
   the engine ~1108ns (SWDGE ucode launch; descgen for 128 descs is
   only ~0.5ns/desc on top) plus ~310ns SEQ dispatch gap -> 240 ops
   x ~1.42us = ~340us serial.  Actual DMA transfer time is only
   ~128us (16.7ns per 4x256B packet) and hides completely under the
   launch serialization ("dynamic dma busy 98%" counts queue-occupied
   time, not transfers -- software_dynamic_dma_active_time is truth).
 - One offset per partition per op is a hard ceiling (128 rows/op):
   multi-offset vector-indirect gathers (offset AP [128, n]) are NOT
   honored by this ucode -- a 2D dest fuses into one descriptor per
   partition (gathers n contiguous rows from offset 0: wrong), and a
   3D dest produces partial writes (NaN) with BOTH 200B-misaligned
   (~25us/op) and 256B-aligned padded-slab (~39us/op) layouts
   (SUNDA-generation DGE only supports indirection_dim=0).
 - Indirect DMA is SWDGE-only: routing the same InstDMACopy+dynamic_ap
   through the idle SP/Activation HWDGE dynamic queues (qActDynamicHW)
   compiles but crashes the runtime -- RTL DGE cannot do indirection.
 - Splitting ops across qPoolDynamic1 (num_swdge_queues=2) executes
   correctly but is ~18% slower (one Q7 pair services both queues);
   single_packet=True is accepted and harmless but changes nothing.
 - The Anthropic extended-ISA ucode ops (dma_gather/ap_gather/
   scatter_add, int16-indexed, 4 parallel SWDGE queue pairs) would
   break the launch bottleneck but crash this runtime (stock NRT
   ucode without the extended opcodes): NRT_EXEC_UNIT_UNRECOVERABLE,
   while the same program passes CoreSim bit-exactly.
"""

import sys

sys.path.insert(0, "/opt/trn_rl_repo")

import numpy as np
import ml_dtypes

# ---- problem constants (hardcoded; kernel.py must be self-contained) ----
B = 16384  # batch
CTX = 8  # context words per row
K1 = 6  # targets per row
DIM = 100  # embedding dim
NW = 200001  # word rows (incl. padding row)
ND = 1000000  # doc rows
NCORES = 8
BPC = B // NCORES  # 2048 batch rows per core
P = 128  # SBUF partitions
TILES = BPC // P  # 16 tiles of 128 rows per core
JPT = CTX + 1 + K1  # 15 gathered rows per batch row
DPAD = 128  # padded row length (256B in bf16)

_CACHE: dict = {}


def _build_program(nrows=NW + ND + NW):
    import concourse.bass as bass
    import concourse.bacc as bacc
    import concourse.mybir as mybir
    import concourse.tile as tile

    bf16 = mybir.dt.bfloat16
    nc = bacc.Bacc("TRN2", target_bir_lowering=False, debug=False,
                   num_devices=NCORES)
    idx_d = nc.dram_tensor("idx", [BPC, JPT], mybir.dt.int32,
                           kind="ExternalInput")
    tab_d = nc.dram_tensor("table", [nrows, DPAD], bf16,
                           kind="ExternalInput")
    out_d = nc.dram_tensor("scores", [BPC, K1], mybir.dt.float32,
                           kind="ExternalOutput")

    with tile.TileContext(nc) as tc:
        with tc.tile_pool(name="sb", bufs=1) as sb, \
             tc.tile_pool(name="scr", bufs=4) as scr:
            idx_sb = sb.tile([P, TILES * JPT], mybir.dt.int32)
            # Load tile 0's indices first so the gather pipeline starts
            # without waiting for the whole index array.
            idx_r = idx_d.ap().rearrange("(t p) j -> p t j", p=P)
            idx_v = idx_sb[:].rearrange("p (t j) -> p t j", t=TILES)
            nc.sync.dma_start(out=idx_v[:, 0:1], in_=idx_r[:, 0:1])
            nc.sync.dma_start(out=idx_v[:, 1:TILES], in_=idx_r[:, 1:TILES])
            scores_sb = sb.tile([P, TILES * K1], mybir.dt.float32)
            # ALL gather destinations statically allocated (61KB/partition
            # fits SBUF easily).  With no buffer reuse there are no
            # write-after-read waits on the Pool engine, so the 240 SWDGE
            # launches (the ~1us fixed cost each is the structural floor
            # of this kernel) issue back-to-back with minimal gaps.
            Gall = sb.tile([P, TILES * (CTX + 1) * DPAD], bf16)
            Gtall = sb.tile([P, TILES * K1 * DPAD], bf16)
            for t in range(TILES):
                # HW indirect DMA supports ONE offset per partition, so we
                # gather the 15 rows of this 128-row batch tile with 15 ops.
                # ctx+doc rows (j<9) and target rows (j>=9) go to separate
                # regions so the h-reduce only waits on the first 9 gathers.
                G = Gall[:, t * (CTX + 1) * DPAD:(t + 1) * (CTX + 1) * DPAD]
                Gt = Gtall[:, t * K1 * DPAD:(t + 1) * K1 * DPAD]
                for j in range(JPT):
                    dst = G[:, j * DPAD:(j + 1) * DPAD] if j <= CTX \
                        else Gt[:, (j - CTX - 1) * DPAD:(j - CTX) * DPAD]
                    nc.gpsimd.indirect_dma_start(
                        out=dst,
                        out_offset=None,
                        in_=tab_d.ap(),
                        in_offset=bass.IndirectOffsetOnAxis(
                            ap=idx_sb[:, t * JPT + j:t * JPT + j + 1],
                            axis=0,
                        ),
                    )
                G3 = G.rearrange("p (j d) -> p d j", j=CTX + 1, d=DPAD)
                h = scr.tile([P, DIM], bf16, tag="h")
                # h = sum of the 8 ctx rows + 1 doc row (slabs j=0..8).
                # DVE accumulates in fp32 internally; bf16 storage only
                # rounds the 9-term sum once (rel ~2^-9, well under the
                # 2e-2 gate) and keeps the 2x 16-bit DVE mode.
                with nc.allow_low_precision(reason="bf16 h, fp32 internal"):
                    nc.vector.tensor_reduce(
                        out=h[:], in_=G3[:, 0:DIM, 0:CTX + 1],
                        axis=mybir.AxisListType.X, op=mybir.AluOpType.add,
                    )
                # prod[p, k, d] = h[p, d] * tgt_k[p, d]; then reduce over d
                prod = scr.tile([P, K1 * DIM], bf16, tag="prod")
                tgt = Gt.rearrange("p (k d) -> p k d", k=K1)
                nc.vector.tensor_tensor(
                    out=prod[:].rearrange("p (k d) -> p k d", k=K1),
                    in0=tgt[:, :, 0:DIM],
                    in1=h[:].unsqueeze(1).to_broadcast([P, K1, DIM]),
                    op=mybir.AluOpType.mult,
                )
                nc.vector.tensor_reduce(
                    out=scores_sb[:, t * K1:(t + 1) * K1],
                    in_=prod[:].rearrange("p (k d) -> p k d", k=K1),
                    axis=mybir.AxisListType.X, op=mybir.AluOpType.add,
                )
                # Flush finished quarters early so the final output DMA
                # isn't one serialized tail after the last vector op.
                if (t + 1) % (TILES // 4) == 0:
                    lo, hi = t + 1 - TILES // 4, t + 1
                    nc.sync.dma_start(
                        out=out_d.ap().rearrange("(t p) k -> p t k", p=P)
                            [:, lo:hi],
                        in_=scores_sb[:].rearrange("p (t k) -> p t k", t=TILES)
                            [:, lo:hi],
                    )
    nc.compile()
    return nc


def _get_program():
    if "nc" not in _CACHE:
        _CACHE["nc"] = _build_program()
    return _CACHE["nc"]


def _pack_inputs(ctxs, docs, targets, D, W, WP):
    """Fuse tables into one 256B-row bf16 table; pack indices to [B,15]."""
    table = np.zeros((NW + ND + NW, DPAD), dtype=ml_dtypes.bfloat16)
    table[:NW, :DIM] = np.asarray(W, dtype=np.float32)
    table[NW:NW + ND, :DIM] = np.asarray(D, dtype=np.float32)
    table[NW + ND:, :DIM] = np.asarray(WP, dtype=np.float32).T
    idx = np.empty((B, JPT), dtype=np.int32)
    idx[:, :CTX] = np.asarray(ctxs, dtype=np.int32)
    idx[:, CTX] = np.asarray(docs, dtype=np.int32) + NW
    idx[:, CTX + 1:] = np.asarray(targets, dtype=np.int32) + (NW + ND)
    return table, idx


def kernel(ctxs, docs, targets, D, W, WP, _trace=False):
    from concourse.bass_utils import run_bass_kernel_spmd

    table, idx = _pack_inputs(ctxs, docs, targets, D, W, WP)
    nc = _get_program()
    in_maps = [
        {"idx": np.ascontiguousarray(idx[c * BPC:(c + 1) * BPC]),
         "table": table}
        for c in range(NCORES)
    ]
    res = run_bass_kernel_spmd(nc, in_maps, core_ids=list(range(NCORES)),
                               trace=_trace)
    out = np.concatenate([res.results[c]["scores"] for c in range(NCORES)],
                         axis=0)
    if _trace:
        return out, res
    return out

